# revision 11
# baseline (speedup 1.0000x reference)
"""MLAFormer Trainium2 kernel.

Sharding: 8 cores = 2 batches x 4 head-pairs. Core c handles batch c//4 and
heads {2*(c%4), 2*(c%4)+1}. Each core computes its partial of the final
projection (its 2 heads' x1/x2 rows plus a 128-row slice of the x rows of
wproj); the host sums the 4 partials per batch.

On-chip layout convention: [features, tokens] (features on partitions).
Inputs arrive [tokens, features] and are transposed on the PE via identity
matmuls. All biases are exact: bq/bk fold in as per-partition adds after the
projection psum; bv folds in after softmax normalization (rows of softmax sum
to 1); bproj is supplied as zeros to 3 of the 4 cores per batch.

Softmax denominators come for free from the attn@v matmul: the v operand
carries an appended ones column (M=65), so psum row 64 accumulates sum_k p.
"""

import numpy as np
import ml_dtypes

import concourse.bass as bass
import concourse.mybir as mybir
import concourse.tile as tile
from concourse import bacc
from concourse.masks import make_identity

B, L, DIM, HEADS, D = 2, 1024, 512, 8, 64
FORE, POST = 256, 1024
LF, LP = 4096, 256
P = 128
NCORES = 8
SCALE = D ** -0.5

F32 = mybir.dt.float32
F32R = mybir.dt.float32r
BF16 = mybir.dt.bfloat16
AF = mybir.ActivationFunctionType
ALU = mybir.AluOpType


def build_nc():
    nc = bacc.Bacc("TRN2", target_bir_lowering=False, debug=False)
    inp = dict(kind="ExternalInput")
    xb = nc.dram_tensor("xb", (L, DIM), F32, **inp).ap()
    fore = nc.dram_tensor("fore", (LF, FORE), F32, **inp).ap()
    post = nc.dram_tensor("post", (LP, POST), F32, **inp).ap()
    wq = nc.dram_tensor("wq", (DIM, P), F32R, **inp).ap()
    bq = nc.dram_tensor("bq", (P, 1), F32, **inp).ap()
    wk1 = nc.dram_tensor("wk1", (FORE, P), BF16, **inp).ap()
    bk1 = nc.dram_tensor("bk1", (P, 1), F32, **inp).ap()
    wv1 = nc.dram_tensor("wv1", (FORE, P), BF16, **inp).ap()
    wk2 = nc.dram_tensor("wk2", (POST, P), BF16, **inp).ap()
    bk2 = nc.dram_tensor("bk2", (P, 1), F32, **inp).ap()
    wv2 = nc.dram_tensor("wv2", (POST, P), BF16, **inp).ap()
    wpx = nc.dram_tensor("wpx", (P, DIM), F32R, **inp).ap()
    wp1 = nc.dram_tensor("wp1", (P, DIM), F32R, **inp).ap()
    wp2 = nc.dram_tensor("wp2", (P, DIM), F32R, **inp).ap()
    bp = nc.dram_tensor("bp", (P, 4), F32, **inp).ap()
    onesd = nc.dram_tensor("onesd", (1, 64), F32R, **inp).ap()
    outT = nc.dram_tensor("outT", (4, P, L), F32, kind="ExternalOutput").ap()

    with tile.TileContext(nc) as tc, nc.allow_low_precision(
            reason="float32r rounding of matmul operands is intentional"):
        with (
            tc.tile_pool(name="const", bufs=1) as const,
            tc.tile_pool(name="big", bufs=1) as big,
            tc.tile_pool(name="work", bufs=3) as work,
            tc.tile_pool(name="pexp", bufs=4) as pexp,
        ):
            ident = const.tile([P, P], F32)
            make_identity(nc, ident)
            ones64 = const.tile([1, 64], F32R)
            nc.sync.dma_start(out=ones64, in_=onesd)

            # persistent activations
            xT = big.tile([P, 4, L], F32R, tag="xT")
            foreT = big.tile([P, 2, LF], BF16, tag="foreT")
            postT = big.tile([P, 8, LP], BF16, tag="postT")
            qT = big.tile([P, L], F32R, tag="qT")
            k1T = big.tile([P, LF], F32R, tag="k1T")
            k2T = big.tile([P, LP], F32R, tag="k2T")
            v1n = big.tile([P, LF // P, 2, 65], BF16, tag="v1n")
            v2n = big.tile([P, LP // P, 2, 65], BF16, tag="v2n")
            x1T = big.tile([P, L], F32R, tag="x1T")
            x2T = big.tile([P, L], F32R, tag="x2T")

            # weights / biases
            wq_sb = const.tile([P, 4, P], F32R, tag="wq")
            nc.sync.dma_start(out=wq_sb, in_=wq.rearrange("(c p) m -> p c m", p=P))
            wk1_sb = const.tile([P, 2, P], BF16, tag="wk1")
            nc.sync.dma_start(out=wk1_sb, in_=wk1.rearrange("(c p) m -> p c m", p=P))
            wv1_sb = const.tile([P, 2, P], BF16, tag="wv1")
            nc.sync.dma_start(out=wv1_sb, in_=wv1.rearrange("(c p) m -> p c m", p=P))
            wk2_sb = const.tile([P, 8, P], BF16, tag="wk2")
            nc.sync.dma_start(out=wk2_sb, in_=wk2.rearrange("(c p) m -> p c m", p=P))
            wv2_sb = const.tile([P, 8, P], BF16, tag="wv2")
            nc.sync.dma_start(out=wv2_sb, in_=wv2.rearrange("(c p) m -> p c m", p=P))
            wpx_sb = const.tile([P, DIM], F32R, tag="wpx")
            nc.sync.dma_start(out=wpx_sb, in_=wpx)
            wp1_sb = const.tile([P, DIM], F32R, tag="wp1")
            nc.sync.dma_start(out=wp1_sb, in_=wp1)
            wp2_sb = const.tile([P, DIM], F32R, tag="wp2")
            nc.sync.dma_start(out=wp2_sb, in_=wp2)
            bias_sb = {}
            for nm, ap in [("bq", bq), ("bk1", bk1), ("bk2", bk2)]:
                t = const.tile([P, 1], F32, tag=nm)
                nc.sync.dma_start(out=t, in_=ap)
                bias_sb[nm] = t
            bp_sb = const.tile([P, 4], F32, tag="bp")
            nc.sync.dma_start(out=bp_sb, in_=bp)

            def copy(out, in_):
                nc.vector.tensor_copy(out=out, in_=in_)

            # ---- Phase A: transposes -------------------------------------
            with (
                tc.tile_pool(name="ps_tr", bufs=4, space="PSUM") as ps_tr,
                tc.tile_pool(name="ps_pj", bufs=2, space="PSUM") as ps_pj,
                tc.tile_pool(name="ps_vn", bufs=2, space="PSUM") as ps_vn,
            ):
                def tr(dst, src):
                    ps = ps_tr.tile([P, P], F32, tag="tr")
                    nc.tensor.transpose(ps, src, ident)
                    copy(dst, ps)

                for t in range(L // P):
                    xa = work.tile([P, DIM], F32, tag="xa")
                    nc.sync.dma_start(out=xa, in_=xb[t * P:(t + 1) * P, :])
                    for f in range(4):
                        tr(xT[:, f, t * P:(t + 1) * P], xa[:, f * P:(f + 1) * P])
                for t in range(LF // P):
                    fa = work.tile([P, FORE], F32, tag="fa")
                    nc.sync.dma_start(out=fa, in_=fore[t * P:(t + 1) * P, :])
                    for f in range(2):
                        tr(foreT[:, f, t * P:(t + 1) * P], fa[:, f * P:(f + 1) * P])
                for t in range(LP // P):
                    pa = work.tile([P, POST], F32, tag="pa")
                    nc.sync.dma_start(out=pa, in_=post[t * P:(t + 1) * P, :])
                    for f in range(8):
                        tr(postT[:, f, t * P:(t + 1) * P], pa[:, f * P:(f + 1) * P])

                # ---- Phase B: q/k projections ----------------------------
                for n in range(2):
                    ps = ps_pj.tile([P, 512], F32, tag="pj")
                    for kc in range(4):
                        nc.tensor.matmul(
                            ps, wq_sb[:, kc, :],
                            xT[:, kc, n * 512:(n + 1) * 512],
                            start=(kc == 0), stop=(kc == 3))
                    nc.vector.tensor_scalar(
                        qT[:, n * 512:(n + 1) * 512], ps, bias_sb["bq"], None,
                        ALU.add)
                for n in range(LF // 512):
                    ps = ps_pj.tile([P, 512], F32, tag="pj")
                    for kc in range(2):
                        nc.tensor.matmul(
                            ps, wk1_sb[:, kc, :],
                            foreT[:, kc, n * 512:(n + 1) * 512],
                            start=(kc == 0), stop=(kc == 1))
                    nc.vector.tensor_scalar(
                        k1T[:, n * 512:(n + 1) * 512], ps, bias_sb["bk1"], None,
                        ALU.add)
                ps = ps_pj.tile([P, 512], F32, tag="pj")
                for kc in range(8):
                    nc.tensor.matmul(
                        ps[:, :LP], wk2_sb[:, kc, :], postT[:, kc, :],
                        start=(kc == 0), stop=(kc == 7))
                nc.vector.tensor_scalar(k2T, ps[:, :LP], bias_sb["bk2"], None,
                                        ALU.add)

                # ---- Phase C: v (natural layout, with ones column) -------
                nc.vector.memset(v1n[:, :, :, 64:65], 1.0)
                nc.vector.memset(v2n[:, :, :, 64:65], 1.0)
                for c in range(LF // P):
                    ps = ps_vn.tile([P, P], F32, tag="vn")
                    for kc in range(2):
                        nc.tensor.matmul(
                            ps, foreT[:, kc, c * P:(c + 1) * P], wv1_sb[:, kc, :],
                            start=(kc == 0), stop=(kc == 1))
                    copy(v1n[:, c, :, 0:64],
                         ps.rearrange("p (h d) -> p h d", h=2))
                for c in range(LP // P):
                    ps = ps_vn.tile([P, P], F32, tag="vn")
                    for kc in range(8):
                        nc.tensor.matmul(
                            ps, postT[:, kc, c * P:(c + 1) * P], wv2_sb[:, kc, :],
                            start=(kc == 0), stop=(kc == 7))
                    copy(v2n[:, c, :, 0:64],
                         ps.rearrange("p (h d) -> p h d", h=2))

            # ---- Phase D: attention ----------------------------------------
            with (
                tc.tile_pool(name="ps_sc", bufs=2, space="PSUM") as ps_sc,
                tc.tile_pool(name="ps_ao", bufs=1, space="PSUM") as ps_ao,
            ):
                def attn(kT, vn, nchunks, x_out, bv):
                    aos = [ps_ao.tile([P, L], F32, tag=f"ao{h}", name=f"ao{h}")
                           for h in range(2)]
                    for c in range(nchunks):
                        cs = slice(c * P, (c + 1) * P)
                        scs = [ps_sc.tile([P, L], F32, tag="sc", name=f"sc{c}_{h}")
                               for h in range(2)]
                        # heads in adjacent row-groups (K=64 at partition 0/64)
                        # -> concurrent on the PE array
                        for n in range(2):
                            ns = slice(n * 512, (n + 1) * 512)
                            for h in range(2):
                                hs = slice(h * 64, (h + 1) * 64)
                                nc.tensor.matmul(
                                    scs[h][:, ns], kT[hs, cs], qT[hs, ns],
                                    start=True, stop=True)
                        pbs = []
                        for h in range(2):
                            pb = pexp.tile([P, L], BF16, tag="pb")
                            nc.scalar.activation(pb, scs[h], AF.Exp)
                            pbs.append(pb)
                        for h in range(2):
                            for n in range(2):
                                ns = slice(n * 512, (n + 1) * 512)
                                nc.tensor.matmul(
                                    aos[h][0:65, ns], vn[:, c, h, :],
                                    pbs[h][:, ns],
                                    start=(c == 0), stop=(c == nchunks - 1))
                    for h in range(2):
                        hs = slice(h * 64, (h + 1) * 64)
                        cp = work.tile([P, L], F32, tag="cp")
                        nc.vector.tensor_copy(out=cp[0:65, :],
                                              in_=aos[h][0:65, :])
                        rc = work.tile([1, L], F32R, tag="rc")
                        nc.vector.reciprocal(out=rc, in_=cp[64:65, :])
                        bc = ps_sc.tile([P, L], F32, tag="sc")
                        for n in range(2):
                            ns = slice(n * 512, (n + 1) * 512)
                            nc.tensor.matmul(bc[0:64, ns], ones64, rc[:, ns],
                                             start=True, stop=True)
                        nc.vector.tensor_tensor(
                            x_out[hs, :], cp[0:64, :], bc[0:64, :], ALU.mult)

                attn(k1T, v1n, LF // P, x1T, None)
                attn(k2T, v2n, LP // P, x2T, None)

            # ---- Phase E: final projection (partial) -----------------------
            with tc.tile_pool(name="ps_f", bufs=4, space="PSUM") as ps_f:
                for m in range(4):
                    ms = slice(m * P, (m + 1) * P)
                    for n in range(2):
                        ns = slice(n * 512, (n + 1) * 512)
                        ps = ps_f.tile([P, 512], F32, tag="f")
                        nc.tensor.matmul(ps, wpx_sb[:, ms], xT[:, 0, ns],
                                         start=True, stop=False)
                        nc.tensor.matmul(ps, wp1_sb[:, ms], x1T[:, ns],
                                         start=False, stop=False)
                        nc.tensor.matmul(ps, wp2_sb[:, ms], x2T[:, ns],
                                         start=False, stop=True)
                        ob = work.tile([P, 512], F32, tag="ob")
                        nc.vector.tensor_scalar(ob, ps, bp_sb[:, m:m + 1], None,
                                                ALU.add)
                        nc.sync.dma_start(out=outT[m, :, ns], in_=ob)

    nc.compile()
    return nc


def make_in_maps(x, fore_x, post_x, wq, bq, wkv1, bkv1, wkv2, bkv2, wproj,
                 bproj):
    bf = ml_dtypes.bfloat16
    in_maps = []
    for c in range(NCORES):
        b, hp = c // 4, c % 4
        cs = slice(hp * P, (hp + 1) * P)
        x_b = np.ascontiguousarray(x[b])
        # rotate x columns so this core's wproj x-slice sits at feature chunk 0
        x_rot = np.ascontiguousarray(np.roll(x_b, -hp * P, axis=1))
        wq_c = np.ascontiguousarray(np.roll(wq[:, cs] * SCALE, -hp * P, axis=0))
        bv1_c = bkv1[512 + hp * P:512 + (hp + 1) * P]
        bv2_c = bkv2[512 + hp * P:512 + (hp + 1) * P]
        bp_eff = ((bproj if hp == 0 else 0.0)
                  + wproj[512 + hp * P:512 + (hp + 1) * P, :].T @ bv1_c
                  + wproj[1024 + hp * P:1024 + (hp + 1) * P, :].T @ bv2_c)
        m = {
            "xb": x_rot.astype(np.float32),
            "fore": np.ascontiguousarray(fore_x[b]).astype(np.float32),
            "post": np.ascontiguousarray(post_x[b]).astype(np.float32),
            "wq": wq_c.astype(np.float32),
            "bq": (bq[cs] * SCALE).reshape(P, 1).astype(np.float32),
            "wk1": np.ascontiguousarray(wkv1[:, cs]).astype(bf),
            "bk1": bkv1[cs].reshape(P, 1).astype(np.float32),
            "wv1": np.ascontiguousarray(wkv1[:, 512 + hp * P:512 + (hp + 1) * P]).astype(bf),
            "wk2": np.ascontiguousarray(wkv2[:, cs]).astype(bf),
            "bk2": bkv2[cs].reshape(P, 1).astype(np.float32),
            "wv2": np.ascontiguousarray(wkv2[:, 512 + hp * P:512 + (hp + 1) * P]).astype(bf),
            "wpx": np.ascontiguousarray(wproj[hp * P:(hp + 1) * P, :]).astype(np.float32),
            "wp1": np.ascontiguousarray(wproj[512 + hp * P:512 + (hp + 1) * P, :]).astype(np.float32),
            "wp2": np.ascontiguousarray(wproj[1024 + hp * P:1024 + (hp + 1) * P, :]).astype(np.float32),
            "bp": bp_eff.reshape(4, P).T.astype(np.float32),
            "onesd": np.ones((1, 64), np.float32),
        }
        in_maps.append(m)
    return in_maps


def gather(results):
    out = np.zeros((B, L, DIM), np.float32)
    for c, res in enumerate(results):
        b = c // 4
        out[b] += res["outT"].reshape(DIM, L).T
    return out


_NC_CACHE = {}


def kernel(**inputs):
    from concourse import bass_utils
    if "nc" not in _NC_CACHE:
        _NC_CACHE["nc"] = build_nc()
    nc = _NC_CACHE["nc"]
    in_maps = make_in_maps(**{k: np.asarray(v) for k, v in inputs.items()})
    res = bass_utils.run_bass_kernel_spmd(nc, in_maps,
                                          core_ids=list(range(NCORES)))
    return gather(res.results)


# revision 12
# speedup vs baseline: 1.0126x; 1.0126x over previous
"""MLAFormer Trainium2 kernel.

Sharding: 8 cores = 2 batches x 4 head-pairs. Core c handles batch c//4 and
heads {2*(c%4), 2*(c%4)+1}. Each core computes its partial of the final
projection (its 2 heads' x1/x2 rows plus a 128-row slice of the x rows of
wproj); the host sums the 4 partials per batch.

On-chip layout convention: [features, tokens] (features on partitions).
Inputs arrive [tokens, features] and are transposed on the PE via identity
matmuls. All biases are exact: bq/bk fold in as per-partition adds after the
projection psum; bv folds in after softmax normalization (rows of softmax sum
to 1); bproj is supplied as zeros to 3 of the 4 cores per batch.

Softmax denominators come for free from the attn@v matmul: the v operand
carries an appended ones column (M=65), so psum row 64 accumulates sum_k p.
"""

import numpy as np
import ml_dtypes

import concourse.bass as bass
import concourse.mybir as mybir
import concourse.tile as tile
from concourse import bacc
from concourse.masks import make_identity

B, L, DIM, HEADS, D = 2, 1024, 512, 8, 64
FORE, POST = 256, 1024
LF, LP = 4096, 256
P = 128
NCORES = 8
SCALE = D ** -0.5

F32 = mybir.dt.float32
F32R = mybir.dt.float32r
BF16 = mybir.dt.bfloat16
AF = mybir.ActivationFunctionType
ALU = mybir.AluOpType


def build_nc():
    nc = bacc.Bacc("TRN2", target_bir_lowering=False, debug=False)
    inp = dict(kind="ExternalInput")
    xb = nc.dram_tensor("xb", (L, DIM), F32, **inp).ap()
    fore = nc.dram_tensor("fore", (LF, FORE), F32, **inp).ap()
    post = nc.dram_tensor("post", (LP, POST), F32, **inp).ap()
    wq = nc.dram_tensor("wq", (DIM, P), F32R, **inp).ap()
    bq = nc.dram_tensor("bq", (P, 1), F32, **inp).ap()
    wk1 = nc.dram_tensor("wk1", (FORE, P), BF16, **inp).ap()
    bk1 = nc.dram_tensor("bk1", (P, 1), F32, **inp).ap()
    wv1 = nc.dram_tensor("wv1", (FORE, P), BF16, **inp).ap()
    wk2 = nc.dram_tensor("wk2", (POST, P), BF16, **inp).ap()
    bk2 = nc.dram_tensor("bk2", (P, 1), F32, **inp).ap()
    wv2 = nc.dram_tensor("wv2", (POST, P), BF16, **inp).ap()
    wpx = nc.dram_tensor("wpx", (P, DIM), F32R, **inp).ap()
    wp1 = nc.dram_tensor("wp1", (P, DIM), F32R, **inp).ap()
    wp2 = nc.dram_tensor("wp2", (P, DIM), F32R, **inp).ap()
    bp = nc.dram_tensor("bp", (P, 4), F32, **inp).ap()
    onesd = nc.dram_tensor("onesd", (1, 64), F32R, **inp).ap()
    outT = nc.dram_tensor("outT", (4, P, L), F32, kind="ExternalOutput").ap()

    with tile.TileContext(nc) as tc, nc.allow_low_precision(
            reason="float32r rounding of matmul operands is intentional"):
        with (
            tc.tile_pool(name="const", bufs=1) as const,
            tc.tile_pool(name="big", bufs=1) as big,
            tc.tile_pool(name="work", bufs=3) as work,
            tc.tile_pool(name="pexp", bufs=4) as pexp,
        ):
            ident = const.tile([P, P], F32)
            make_identity(nc, ident)
            ones64 = const.tile([1, 64], F32R)
            nc.sync.dma_start(out=ones64, in_=onesd)

            # persistent activations
            xT = big.tile([P, 4, L], F32R, tag="xT")
            foreT = big.tile([P, 2, LF], BF16, tag="foreT")
            postT = big.tile([P, 8, LP], BF16, tag="postT")
            qT = big.tile([P, L], F32R, tag="qT")
            k1T = big.tile([P, LF], F32R, tag="k1T")
            k2T = big.tile([P, LP], F32R, tag="k2T")
            v1n = big.tile([P, LF // P, 2, 65], BF16, tag="v1n")
            v2n = big.tile([P, LP // P, 2, 65], BF16, tag="v2n")
            x1T = big.tile([P, L], F32R, tag="x1T")
            x2T = big.tile([P, L], F32R, tag="x2T")

            # weights / biases
            wq_sb = const.tile([P, 4, P], F32R, tag="wq")
            nc.sync.dma_start(out=wq_sb, in_=wq.rearrange("(c p) m -> p c m", p=P))
            wk1_sb = const.tile([P, 2, P], BF16, tag="wk1")
            nc.sync.dma_start(out=wk1_sb, in_=wk1.rearrange("(c p) m -> p c m", p=P))
            wv1_sb = const.tile([P, 2, P], BF16, tag="wv1")
            nc.sync.dma_start(out=wv1_sb, in_=wv1.rearrange("(c p) m -> p c m", p=P))
            wk2_sb = const.tile([P, 8, P], BF16, tag="wk2")
            nc.sync.dma_start(out=wk2_sb, in_=wk2.rearrange("(c p) m -> p c m", p=P))
            wv2_sb = const.tile([P, 8, P], BF16, tag="wv2")
            nc.sync.dma_start(out=wv2_sb, in_=wv2.rearrange("(c p) m -> p c m", p=P))
            wpx_sb = const.tile([P, DIM], F32R, tag="wpx")
            nc.sync.dma_start(out=wpx_sb, in_=wpx)
            wp1_sb = const.tile([P, DIM], F32R, tag="wp1")
            nc.sync.dma_start(out=wp1_sb, in_=wp1)
            wp2_sb = const.tile([P, DIM], F32R, tag="wp2")
            nc.sync.dma_start(out=wp2_sb, in_=wp2)
            bias_sb = {}
            for nm, ap in [("bq", bq), ("bk1", bk1), ("bk2", bk2)]:
                t = const.tile([P, 1], F32, tag=nm)
                nc.sync.dma_start(out=t, in_=ap)
                bias_sb[nm] = t
            bp_sb = const.tile([P, 4], F32, tag="bp")
            nc.sync.dma_start(out=bp_sb, in_=bp)

            def copy(out, in_):
                nc.vector.tensor_copy(out=out, in_=in_)

            # ---- Phase A: transposes -------------------------------------
            with (
                tc.tile_pool(name="ps_tr", bufs=4, space="PSUM") as ps_tr,
                tc.tile_pool(name="ps_pj", bufs=2, space="PSUM") as ps_pj,
                tc.tile_pool(name="ps_vn", bufs=2, space="PSUM") as ps_vn,
            ):
                trcnt = [0]

                def tr(dst, src):
                    ps = ps_tr.tile([P, P], F32, tag="tr")
                    nc.tensor.transpose(ps, src, ident)
                    if trcnt[0] % 2 == 0:
                        nc.vector.tensor_copy(out=dst, in_=ps)
                    else:
                        nc.scalar.copy(out=dst, in_=ps)
                    trcnt[0] += 1

                for t in range(L // P):
                    xa = work.tile([P, DIM], F32, tag="xa")
                    nc.sync.dma_start(out=xa, in_=xb[t * P:(t + 1) * P, :])
                    for f in range(4):
                        tr(xT[:, f, t * P:(t + 1) * P], xa[:, f * P:(f + 1) * P])
                for t in range(LF // P):
                    fa = work.tile([P, FORE], F32, tag="fa")
                    nc.sync.dma_start(out=fa, in_=fore[t * P:(t + 1) * P, :])
                    for f in range(2):
                        tr(foreT[:, f, t * P:(t + 1) * P], fa[:, f * P:(f + 1) * P])
                for t in range(LP // P):
                    pa = work.tile([P, POST], F32, tag="pa")
                    nc.sync.dma_start(out=pa, in_=post[t * P:(t + 1) * P, :])
                    for f in range(8):
                        tr(postT[:, f, t * P:(t + 1) * P], pa[:, f * P:(f + 1) * P])

                # ---- Phase B: q/k projections ----------------------------
                for n in range(2):
                    ps = ps_pj.tile([P, 512], F32, tag="pj")
                    for kc in range(4):
                        nc.tensor.matmul(
                            ps, wq_sb[:, kc, :],
                            xT[:, kc, n * 512:(n + 1) * 512],
                            start=(kc == 0), stop=(kc == 3))
                    nc.vector.tensor_scalar(
                        qT[:, n * 512:(n + 1) * 512], ps, bias_sb["bq"], None,
                        ALU.add)
                for n in range(LF // 512):
                    ps = ps_pj.tile([P, 512], F32, tag="pj")
                    for kc in range(2):
                        nc.tensor.matmul(
                            ps, wk1_sb[:, kc, :],
                            foreT[:, kc, n * 512:(n + 1) * 512],
                            start=(kc == 0), stop=(kc == 1))
                    nc.vector.tensor_scalar(
                        k1T[:, n * 512:(n + 1) * 512], ps, bias_sb["bk1"], None,
                        ALU.add)
                ps = ps_pj.tile([P, 512], F32, tag="pj")
                for kc in range(8):
                    nc.tensor.matmul(
                        ps[:, :LP], wk2_sb[:, kc, :], postT[:, kc, :],
                        start=(kc == 0), stop=(kc == 7))
                nc.vector.tensor_scalar(k2T, ps[:, :LP], bias_sb["bk2"], None,
                                        ALU.add)

                # ---- Phase C: v (natural layout, with ones column) -------
                nc.vector.memset(v1n[:, :, :, 64:65], 1.0)
                nc.vector.memset(v2n[:, :, :, 64:65], 1.0)
                for c in range(LF // P):
                    ps = ps_vn.tile([P, P], F32, tag="vn")
                    for kc in range(2):
                        nc.tensor.matmul(
                            ps, foreT[:, kc, c * P:(c + 1) * P], wv1_sb[:, kc, :],
                            start=(kc == 0), stop=(kc == 1))
                    copy(v1n[:, c, :, 0:64],
                         ps.rearrange("p (h d) -> p h d", h=2))
                for c in range(LP // P):
                    ps = ps_vn.tile([P, P], F32, tag="vn")
                    for kc in range(8):
                        nc.tensor.matmul(
                            ps, postT[:, kc, c * P:(c + 1) * P], wv2_sb[:, kc, :],
                            start=(kc == 0), stop=(kc == 7))
                    copy(v2n[:, c, :, 0:64],
                         ps.rearrange("p (h d) -> p h d", h=2))

            # ---- Phase D: attention ----------------------------------------
            with (
                tc.tile_pool(name="ps_sc", bufs=2, space="PSUM") as ps_sc,
                tc.tile_pool(name="ps_ao", bufs=1, space="PSUM") as ps_ao,
            ):
                def attn(kT, vn, nchunks, x_out, bv):
                    aos = [ps_ao.tile([P, L], F32, tag=f"ao{h}", name=f"ao{h}")
                           for h in range(2)]
                    for c in range(nchunks):
                        cs = slice(c * P, (c + 1) * P)
                        scs = [ps_sc.tile([P, L], F32, tag="sc", name=f"sc{c}_{h}")
                               for h in range(2)]
                        # heads in adjacent row-groups (K=64 at partition 0/64)
                        # -> concurrent on the PE array
                        for n in range(2):
                            ns = slice(n * 512, (n + 1) * 512)
                            for h in range(2):
                                hs = slice(h * 64, (h + 1) * 64)
                                nc.tensor.matmul(
                                    scs[h][:, ns], kT[hs, cs], qT[hs, ns],
                                    start=True, stop=True)
                        pbs = []
                        for h in range(2):
                            pb = pexp.tile([P, L], BF16, tag="pb")
                            nc.scalar.activation(pb, scs[h], AF.Exp)
                            pbs.append(pb)
                        for h in range(2):
                            for n in range(2):
                                ns = slice(n * 512, (n + 1) * 512)
                                nc.tensor.matmul(
                                    aos[h][0:65, ns], vn[:, c, h, :],
                                    pbs[h][:, ns],
                                    start=(c == 0), stop=(c == nchunks - 1))
                    for h in range(2):
                        hs = slice(h * 64, (h + 1) * 64)
                        cp = work.tile([P, L], F32, tag="cp")
                        nc.vector.tensor_copy(out=cp[0:65, :],
                                              in_=aos[h][0:65, :])
                        rc = work.tile([1, L], F32R, tag="rc")
                        nc.vector.reciprocal(out=rc, in_=cp[64:65, :])
                        bc = ps_sc.tile([P, L], F32, tag="sc")
                        for n in range(2):
                            ns = slice(n * 512, (n + 1) * 512)
                            nc.tensor.matmul(bc[0:64, ns], ones64, rc[:, ns],
                                             start=True, stop=True)
                        nc.vector.tensor_tensor(
                            x_out[hs, :], cp[0:64, :], bc[0:64, :], ALU.mult)

                attn(k1T, v1n, LF // P, x1T, None)
                attn(k2T, v2n, LP // P, x2T, None)

            # ---- Phase E: final projection (partial) -----------------------
            with tc.tile_pool(name="ps_f", bufs=4, space="PSUM") as ps_f:
                for m in range(4):
                    ms = slice(m * P, (m + 1) * P)
                    for n in range(2):
                        ns = slice(n * 512, (n + 1) * 512)
                        ps = ps_f.tile([P, 512], F32, tag="f")
                        nc.tensor.matmul(ps, wpx_sb[:, ms], xT[:, 0, ns],
                                         start=True, stop=False)
                        nc.tensor.matmul(ps, wp1_sb[:, ms], x1T[:, ns],
                                         start=False, stop=False)
                        nc.tensor.matmul(ps, wp2_sb[:, ms], x2T[:, ns],
                                         start=False, stop=True)
                        ob = work.tile([P, 512], F32, tag="ob")
                        nc.vector.tensor_scalar(ob, ps, bp_sb[:, m:m + 1], None,
                                                ALU.add)
                        nc.sync.dma_start(out=outT[m, :, ns], in_=ob)

    nc.compile()
    return nc


def make_in_maps(x, fore_x, post_x, wq, bq, wkv1, bkv1, wkv2, bkv2, wproj,
                 bproj):
    bf = ml_dtypes.bfloat16
    in_maps = []
    for c in range(NCORES):
        b, hp = c // 4, c % 4
        cs = slice(hp * P, (hp + 1) * P)
        x_b = np.ascontiguousarray(x[b])
        # rotate x columns so this core's wproj x-slice sits at feature chunk 0
        x_rot = np.ascontiguousarray(np.roll(x_b, -hp * P, axis=1))
        wq_c = np.ascontiguousarray(np.roll(wq[:, cs] * SCALE, -hp * P, axis=0))
        bv1_c = bkv1[512 + hp * P:512 + (hp + 1) * P]
        bv2_c = bkv2[512 + hp * P:512 + (hp + 1) * P]
        bp_eff = ((bproj if hp == 0 else 0.0)
                  + wproj[512 + hp * P:512 + (hp + 1) * P, :].T @ bv1_c
                  + wproj[1024 + hp * P:1024 + (hp + 1) * P, :].T @ bv2_c)
        m = {
            "xb": x_rot.astype(np.float32),
            "fore": np.ascontiguousarray(fore_x[b]).astype(np.float32),
            "post": np.ascontiguousarray(post_x[b]).astype(np.float32),
            "wq": wq_c.astype(np.float32),
            "bq": (bq[cs] * SCALE).reshape(P, 1).astype(np.float32),
            "wk1": np.ascontiguousarray(wkv1[:, cs]).astype(bf),
            "bk1": bkv1[cs].reshape(P, 1).astype(np.float32),
            "wv1": np.ascontiguousarray(wkv1[:, 512 + hp * P:512 + (hp + 1) * P]).astype(bf),
            "wk2": np.ascontiguousarray(wkv2[:, cs]).astype(bf),
            "bk2": bkv2[cs].reshape(P, 1).astype(np.float32),
            "wv2": np.ascontiguousarray(wkv2[:, 512 + hp * P:512 + (hp + 1) * P]).astype(bf),
            "wpx": np.ascontiguousarray(wproj[hp * P:(hp + 1) * P, :]).astype(np.float32),
            "wp1": np.ascontiguousarray(wproj[512 + hp * P:512 + (hp + 1) * P, :]).astype(np.float32),
            "wp2": np.ascontiguousarray(wproj[1024 + hp * P:1024 + (hp + 1) * P, :]).astype(np.float32),
            "bp": bp_eff.reshape(4, P).T.astype(np.float32),
            "onesd": np.ones((1, 64), np.float32),
        }
        in_maps.append(m)
    return in_maps


def gather(results):
    out = np.zeros((B, L, DIM), np.float32)
    for c, res in enumerate(results):
        b = c // 4
        out[b] += res["outT"].reshape(DIM, L).T
    return out


_NC_CACHE = {}


def kernel(**inputs):
    from concourse import bass_utils
    if "nc" not in _NC_CACHE:
        _NC_CACHE["nc"] = build_nc()
    nc = _NC_CACHE["nc"]
    in_maps = make_in_maps(**{k: np.asarray(v) for k, v in inputs.items()})
    res = bass_utils.run_bass_kernel_spmd(nc, in_maps,
                                          core_ids=list(range(NCORES)))
    return gather(res.results)


# revision 14
# speedup vs baseline: 1.0852x; 1.0717x over previous
"""MLAFormer Trainium2 kernel.

Sharding: 8 cores = 2 batches x 4 head-pairs. Core c handles batch c//4 and
heads {2*(c%4), 2*(c%4)+1}. Each core computes its partial of the final
projection (its 2 heads' x1/x2 rows plus a 128-row slice of the x rows of
wproj); the host sums the 4 partials per batch.

On-chip layout convention: [features, tokens] (features on partitions).
Inputs arrive [tokens, features] and are transposed on the PE via identity
matmuls. All biases are exact: bq/bk fold in as per-partition adds after the
projection psum; bv folds in after softmax normalization (rows of softmax sum
to 1); bproj is supplied as zeros to 3 of the 4 cores per batch.

Softmax denominators come for free from the attn@v matmul: the v operand
carries an appended ones column (M=65), so psum row 64 accumulates sum_k p.
"""

import numpy as np
import ml_dtypes

import concourse.bass as bass
import concourse.mybir as mybir
import concourse.tile as tile
from concourse import bacc
from concourse.masks import make_identity

B, L, DIM, HEADS, D = 2, 1024, 512, 8, 64
FORE, POST = 256, 1024
LF, LP = 4096, 256
P = 128
NCORES = 8
SCALE = D ** -0.5

F32 = mybir.dt.float32
F32R = mybir.dt.float32r
BF16 = mybir.dt.bfloat16
AF = mybir.ActivationFunctionType
ALU = mybir.AluOpType


def build_nc():
    nc = bacc.Bacc("TRN2", target_bir_lowering=False, debug=False)
    inp = dict(kind="ExternalInput")
    xb = nc.dram_tensor("xb", (L, DIM), F32, **inp).ap()
    fore = nc.dram_tensor("fore", (LF, FORE), F32, **inp).ap()
    post = nc.dram_tensor("post", (LP, POST), F32, **inp).ap()
    wq = nc.dram_tensor("wq", (DIM, P), F32R, **inp).ap()
    bq = nc.dram_tensor("bq", (P, 1), F32, **inp).ap()
    wk1 = nc.dram_tensor("wk1", (FORE, P), BF16, **inp).ap()
    bk1 = nc.dram_tensor("bk1", (P, 1), F32, **inp).ap()
    wv1 = nc.dram_tensor("wv1", (FORE, P), BF16, **inp).ap()
    wk2 = nc.dram_tensor("wk2", (POST, P), BF16, **inp).ap()
    bk2 = nc.dram_tensor("bk2", (P, 1), F32, **inp).ap()
    wv2 = nc.dram_tensor("wv2", (POST, P), BF16, **inp).ap()
    wpx = nc.dram_tensor("wpx", (P, DIM), F32R, **inp).ap()
    wp1 = nc.dram_tensor("wp1", (P, DIM), F32R, **inp).ap()
    wp2 = nc.dram_tensor("wp2", (P, DIM), F32R, **inp).ap()
    bp = nc.dram_tensor("bp", (P, 4), F32, **inp).ap()
    onesd = nc.dram_tensor("onesd", (1, 64), F32R, **inp).ap()
    outT = nc.dram_tensor("outT", (4, P, L), F32, kind="ExternalOutput").ap()

    with tile.TileContext(nc) as tc, nc.allow_low_precision(
            reason="float32r rounding of matmul operands is intentional"):
        with (
            tc.tile_pool(name="const", bufs=1) as const,
            tc.tile_pool(name="big", bufs=1) as big,
            tc.tile_pool(name="work", bufs=4) as work,
            tc.tile_pool(name="pexp", bufs=6) as pexp,
        ):
            ident = const.tile([P, P], F32)
            make_identity(nc, ident)
            ones64 = const.tile([1, 64], F32R)
            nc.sync.dma_start(out=ones64, in_=onesd)

            # persistent activations
            xT = big.tile([P, 4, L], F32R, tag="xT")
            foreT = big.tile([P, 2, LF], BF16, tag="foreT")
            postT = big.tile([P, 8, LP], BF16, tag="postT")
            qT = big.tile([P, L], F32R, tag="qT")
            k1T = big.tile([P, LF], F32R, tag="k1T")
            k2T = big.tile([P, LP], F32R, tag="k2T")
            v1n = big.tile([P, LF // P, 2, 65], BF16, tag="v1n")
            v2n = big.tile([P, LP // P, 2, 65], BF16, tag="v2n")
            x1T = big.tile([P, L], F32R, tag="x1T")
            x2T = big.tile([P, L], F32R, tag="x2T")

            # weights / biases
            wq_sb = const.tile([P, 4, P], F32R, tag="wq")
            nc.sync.dma_start(out=wq_sb, in_=wq.rearrange("(c p) m -> p c m", p=P))
            wk1_sb = const.tile([P, 2, P], BF16, tag="wk1")
            nc.sync.dma_start(out=wk1_sb, in_=wk1.rearrange("(c p) m -> p c m", p=P))
            wv1_sb = const.tile([P, 2, P], BF16, tag="wv1")
            nc.sync.dma_start(out=wv1_sb, in_=wv1.rearrange("(c p) m -> p c m", p=P))
            wk2_sb = const.tile([P, 8, P], BF16, tag="wk2")
            nc.sync.dma_start(out=wk2_sb, in_=wk2.rearrange("(c p) m -> p c m", p=P))
            wv2_sb = const.tile([P, 8, P], BF16, tag="wv2")
            nc.sync.dma_start(out=wv2_sb, in_=wv2.rearrange("(c p) m -> p c m", p=P))
            wpx_sb = const.tile([P, DIM], F32R, tag="wpx")
            nc.sync.dma_start(out=wpx_sb, in_=wpx)
            wp1_sb = const.tile([P, DIM], F32R, tag="wp1")
            nc.sync.dma_start(out=wp1_sb, in_=wp1)
            wp2_sb = const.tile([P, DIM], F32R, tag="wp2")
            nc.sync.dma_start(out=wp2_sb, in_=wp2)
            bias_sb = {}
            for nm, ap in [("bq", bq), ("bk1", bk1), ("bk2", bk2)]:
                t = const.tile([P, 1], F32, tag=nm)
                nc.sync.dma_start(out=t, in_=ap)
                bias_sb[nm] = t
            bp_sb = const.tile([P, 4], F32, tag="bp")
            nc.sync.dma_start(out=bp_sb, in_=bp)

            def copy(out, in_):
                nc.vector.tensor_copy(out=out, in_=in_)

            # ---- Phase A: transposes -------------------------------------
            with (
                tc.tile_pool(name="ps_tr", bufs=4, space="PSUM") as ps_tr,
                tc.tile_pool(name="ps_pj", bufs=2, space="PSUM") as ps_pj,
                tc.tile_pool(name="ps_vn", bufs=2, space="PSUM") as ps_vn,
            ):
                trcnt = [0]

                def tr(dst, src):
                    ps = ps_tr.tile([P, P], F32, tag="tr")
                    nc.tensor.transpose(ps, src, ident)
                    if trcnt[0] % 2 == 0:
                        nc.vector.tensor_copy(out=dst, in_=ps)
                    else:
                        nc.scalar.copy(out=dst, in_=ps)
                    trcnt[0] += 1

                for t in range(L // P):
                    xa = work.tile([P, DIM], F32, tag="xa")
                    nc.sync.dma_start(out=xa, in_=xb[t * P:(t + 1) * P, :])
                    for f in range(4):
                        tr(xT[:, f, t * P:(t + 1) * P], xa[:, f * P:(f + 1) * P])
                for t in range(LF // P):
                    fa = work.tile([P, FORE], F32, tag="fa")
                    nc.sync.dma_start(out=fa, in_=fore[t * P:(t + 1) * P, :])
                    for f in range(2):
                        tr(foreT[:, f, t * P:(t + 1) * P], fa[:, f * P:(f + 1) * P])
                for t in range(LP // P):
                    pa = work.tile([P, POST], F32, tag="pa")
                    nc.sync.dma_start(out=pa, in_=post[t * P:(t + 1) * P, :])
                    for f in range(8):
                        tr(postT[:, f, t * P:(t + 1) * P], pa[:, f * P:(f + 1) * P])

                # ---- Phase B: q/k projections ----------------------------
                for n in range(2):
                    ps = ps_pj.tile([P, 512], F32, tag="pj")
                    for kc in range(4):
                        nc.tensor.matmul(
                            ps, wq_sb[:, kc, :],
                            xT[:, kc, n * 512:(n + 1) * 512],
                            start=(kc == 0), stop=(kc == 3))
                    nc.vector.tensor_scalar(
                        qT[:, n * 512:(n + 1) * 512], ps, bias_sb["bq"], None,
                        ALU.add)
                for n in range(LF // 512):
                    ps = ps_pj.tile([P, 512], F32, tag="pj")
                    for kc in range(2):
                        nc.tensor.matmul(
                            ps, wk1_sb[:, kc, :],
                            foreT[:, kc, n * 512:(n + 1) * 512],
                            start=(kc == 0), stop=(kc == 1))
                    nc.vector.tensor_scalar(
                        k1T[:, n * 512:(n + 1) * 512], ps, bias_sb["bk1"], None,
                        ALU.add)
                ps = ps_pj.tile([P, 512], F32, tag="pj")
                for kc in range(8):
                    nc.tensor.matmul(
                        ps[:, :LP], wk2_sb[:, kc, :], postT[:, kc, :],
                        start=(kc == 0), stop=(kc == 7))
                nc.vector.tensor_scalar(k2T, ps[:, :LP], bias_sb["bk2"], None,
                                        ALU.add)

                # ---- Phase C: v (natural layout, with ones column) -------
                nc.vector.memset(v1n[:, :, :, 64:65], 1.0)
                nc.vector.memset(v2n[:, :, :, 64:65], 1.0)
                for c in range(LF // P):
                    ps = ps_vn.tile([P, P], F32, tag="vn")
                    for kc in range(2):
                        nc.tensor.matmul(
                            ps, foreT[:, kc, c * P:(c + 1) * P], wv1_sb[:, kc, :],
                            start=(kc == 0), stop=(kc == 1))
                    copy(v1n[:, c, :, 0:64],
                         ps.rearrange("p (h d) -> p h d", h=2))
                for c in range(LP // P):
                    ps = ps_vn.tile([P, P], F32, tag="vn")
                    for kc in range(8):
                        nc.tensor.matmul(
                            ps, postT[:, kc, c * P:(c + 1) * P], wv2_sb[:, kc, :],
                            start=(kc == 0), stop=(kc == 7))
                    copy(v2n[:, c, :, 0:64],
                         ps.rearrange("p (h d) -> p h d", h=2))

            # ---- Phase D: attention ----------------------------------------
            with (
                tc.tile_pool(name="ps_sc", bufs=2, space="PSUM") as ps_sc,
                tc.tile_pool(name="ps_ao", bufs=1, space="PSUM") as ps_ao,
            ):
                def attn(kT, vn, nchunks, x_out, bv):
                    aos = [ps_ao.tile([P, L], F32, tag=f"ao{h}", name=f"ao{h}")
                           for h in range(2)]
                    for c in range(nchunks):
                        cs = slice(c * P, (c + 1) * P)
                        scs = [ps_sc.tile([P, L], F32, tag="sc", name=f"sc{c}_{h}")
                               for h in range(2)]
                        # heads in adjacent row-groups (K=64 at partition 0/64)
                        # -> concurrent on the PE array
                        for n in range(2):
                            ns = slice(n * 512, (n + 1) * 512)
                            for h in range(2):
                                hs = slice(h * 64, (h + 1) * 64)
                                nc.tensor.matmul(
                                    scs[h][:, ns], kT[hs, cs], qT[hs, ns],
                                    start=True, stop=True)
                        pbs = []
                        for h in range(2):
                            pb = pexp.tile([P, L], BF16, tag="pb")
                            nc.scalar.activation(pb, scs[h], AF.Exp)
                            pbs.append(pb)
                        for h in range(2):
                            for n in range(2):
                                ns = slice(n * 512, (n + 1) * 512)
                                nc.tensor.matmul(
                                    aos[h][0:65, ns], vn[:, c, h, :],
                                    pbs[h][:, ns],
                                    start=(c == 0), stop=(c == nchunks - 1))
                    for h in range(2):
                        hs = slice(h * 64, (h + 1) * 64)
                        cp = work.tile([P, L], F32, tag="cp")
                        nc.vector.tensor_copy(out=cp[0:65, :],
                                              in_=aos[h][0:65, :])
                        rc = work.tile([1, L], F32R, tag="rc")
                        nc.vector.reciprocal(out=rc, in_=cp[64:65, :])
                        bc = ps_sc.tile([P, L], F32, tag="sc")
                        for n in range(2):
                            ns = slice(n * 512, (n + 1) * 512)
                            nc.tensor.matmul(bc[0:64, ns], ones64, rc[:, ns],
                                             start=True, stop=True)
                        nc.vector.tensor_tensor(
                            x_out[hs, :], cp[0:64, :], bc[0:64, :], ALU.mult)

                attn(k1T, v1n, LF // P, x1T, None)
                attn(k2T, v2n, LP // P, x2T, None)

            # ---- Phase E: final projection (partial) -----------------------
            with tc.tile_pool(name="ps_f", bufs=4, space="PSUM") as ps_f:
                for m in range(4):
                    ms = slice(m * P, (m + 1) * P)
                    for n in range(2):
                        ns = slice(n * 512, (n + 1) * 512)
                        ps = ps_f.tile([P, 512], F32, tag="f")
                        nc.tensor.matmul(ps, wpx_sb[:, ms], xT[:, 0, ns],
                                         start=True, stop=False)
                        nc.tensor.matmul(ps, wp1_sb[:, ms], x1T[:, ns],
                                         start=False, stop=False)
                        nc.tensor.matmul(ps, wp2_sb[:, ms], x2T[:, ns],
                                         start=False, stop=True)
                        ob = work.tile([P, 512], F32, tag="ob")
                        nc.vector.tensor_scalar(ob, ps, bp_sb[:, m:m + 1], None,
                                                ALU.add)
                        nc.sync.dma_start(out=outT[m, :, ns], in_=ob)

    nc.compile()
    return nc


def make_in_maps(x, fore_x, post_x, wq, bq, wkv1, bkv1, wkv2, bkv2, wproj,
                 bproj):
    bf = ml_dtypes.bfloat16
    in_maps = []
    for c in range(NCORES):
        b, hp = c // 4, c % 4
        cs = slice(hp * P, (hp + 1) * P)
        x_b = np.ascontiguousarray(x[b])
        # rotate x columns so this core's wproj x-slice sits at feature chunk 0
        x_rot = np.ascontiguousarray(np.roll(x_b, -hp * P, axis=1))
        wq_c = np.ascontiguousarray(np.roll(wq[:, cs] * SCALE, -hp * P, axis=0))
        bv1_c = bkv1[512 + hp * P:512 + (hp + 1) * P]
        bv2_c = bkv2[512 + hp * P:512 + (hp + 1) * P]
        bp_eff = ((bproj if hp == 0 else 0.0)
                  + wproj[512 + hp * P:512 + (hp + 1) * P, :].T @ bv1_c
                  + wproj[1024 + hp * P:1024 + (hp + 1) * P, :].T @ bv2_c)
        m = {
            "xb": x_rot.astype(np.float32),
            "fore": np.ascontiguousarray(fore_x[b]).astype(np.float32),
            "post": np.ascontiguousarray(post_x[b]).astype(np.float32),
            "wq": wq_c.astype(np.float32),
            "bq": (bq[cs] * SCALE).reshape(P, 1).astype(np.float32),
            "wk1": np.ascontiguousarray(wkv1[:, cs]).astype(bf),
            "bk1": bkv1[cs].reshape(P, 1).astype(np.float32),
            "wv1": np.ascontiguousarray(wkv1[:, 512 + hp * P:512 + (hp + 1) * P]).astype(bf),
            "wk2": np.ascontiguousarray(wkv2[:, cs]).astype(bf),
            "bk2": bkv2[cs].reshape(P, 1).astype(np.float32),
            "wv2": np.ascontiguousarray(wkv2[:, 512 + hp * P:512 + (hp + 1) * P]).astype(bf),
            "wpx": np.ascontiguousarray(wproj[hp * P:(hp + 1) * P, :]).astype(np.float32),
            "wp1": np.ascontiguousarray(wproj[512 + hp * P:512 + (hp + 1) * P, :]).astype(np.float32),
            "wp2": np.ascontiguousarray(wproj[1024 + hp * P:1024 + (hp + 1) * P, :]).astype(np.float32),
            "bp": bp_eff.reshape(4, P).T.astype(np.float32),
            "onesd": np.ones((1, 64), np.float32),
        }
        in_maps.append(m)
    return in_maps


def gather(results):
    out = np.zeros((B, L, DIM), np.float32)
    for c, res in enumerate(results):
        b = c // 4
        out[b] += res["outT"].reshape(DIM, L).T
    return out


_NC_CACHE = {}


def kernel(**inputs):
    from concourse import bass_utils
    if "nc" not in _NC_CACHE:
        _NC_CACHE["nc"] = build_nc()
    nc = _NC_CACHE["nc"]
    in_maps = make_in_maps(**{k: np.asarray(v) for k, v in inputs.items()})
    res = bass_utils.run_bass_kernel_spmd(nc, in_maps,
                                          core_ids=list(range(NCORES)))
    return gather(res.results)
